# revision 9
# baseline (speedup 1.0000x reference)
"""Trainium2 Bass kernel for CustomMultiHeadAttention (RoPE + causal MHA).

Sharding: 8 cores = 2 batches x 4 head-groups (4 heads each).
Each core computes, for its (batch, head-group):
  QT/KT = (Wq|Wk col-slice, half-permuted).T @ xT   -> [256, S] feature-major
  RoPE on QT/KT (full-tile DVE ops thanks to half-grouped layout)
  V = xT.T @ Wv col-slice                            -> [S, 256] natural
  per head pair: scoresT[k,q] = KT_h.T @ QT_h (quadrant-packed, K=64)
            expT = exp(scoresT/8), one ACTIVATE covering both heads
            causal: skip blocks above diagonal, 0/1-mask diagonal blocks
            ctxT[d,q] (+denominator rows via ones-columns in V_aug) = V_aug.T @ expT
  normalize ctxT with reciprocal_approx_fast + mult
  partial_out = ctxT.T @ Wo row-slice               -> [S, 1024] bf16
Host: sums the 4 head-group partials per batch, adds bo.

All DRAM traffic is bf16 (x, weights, partial outputs); sin/cos fp32.
Emission is software-pipelined: projections for chunk qc+2 are emitted
between the two attention pairs of chunk qc so the PE always has dense
matmul work while ACT runs exp and DVE runs RoPE/normalize.
"""

import os
import sys

for _p in ("/opt/trn_rl_repo", "/root/.axon_site/_ro/trn_rl_repo"):
    if os.path.isdir(_p) and _p not in sys.path:
        sys.path.insert(0, _p)

import numpy as np
import ml_dtypes

import concourse.bass as bass
import concourse.bacc as bacc
import concourse.mybir as mybir
import concourse.tile as tile
from concourse.bass_utils import run_bass_kernel_spmd

F32 = mybir.dt.float32
BF16 = mybir.dt.bfloat16
AF = mybir.ActivationFunctionType
ALU = mybir.AluOpType

NUM_HEADS = 16
HD = 64
D = NUM_HEADS * HD  # 1024
B = 2
S = 2048
NCORES = 8
HPC = 4            # heads per core
JC = HPC * HD      # 256 per-core projection width
P = 128

RECIP = os.environ.get("KERNEL_RECIP", "fast")   # fast | exact


def build_core(tc, io, s_len=S):
    """Emit the per-core program."""
    nc = tc.nc
    SL = s_len
    NST = SL // P          # 128-row seq tiles
    NQC = SL // 512        # 512-wide q chunks
    NDT = D // P           # 8 k-tiles over d_model
    scale = 1.0 / np.sqrt(HD)

    xT_d, wq_d, wk_d, wv_d, wo_d = io["xT"], io["wq"], io["wk"], io["wv"], io["wo"]
    sin_d, cos_d, mask_d, out_d = io["sin"], io["cos"], io["mask"], io["out"]

    import contextlib
    with contextlib.ExitStack() as ctx:
        cpool = ctx.enter_context(tc.tile_pool(name="const", bufs=1))
        epool = ctx.enter_context(tc.tile_pool(name="expt", bufs=4))
        tpool = ctx.enter_context(tc.tile_pool(name="tmps", bufs=4))
        # PSUM: 8 banks total = sc(2x2) + ctx(1x2) + pr(1x2)
        ps_sc = ctx.enter_context(tc.tile_pool(name="ps_sc", bufs=2, space="PSUM"))
        ps_ctx = ctx.enter_context(tc.tile_pool(name="ps_ctx", bufs=1, space="PSUM"))
        ps_pr = ctx.enter_context(tc.tile_pool(name="ps_pr", bufs=1, space="PSUM"))

        # ---- persistent SBUF tensors ----
        xT = cpool.tile([P, NDT, SL], BF16, tag="xT")
        wq = cpool.tile([P, NDT, JC], BF16, tag="wq")
        wk = cpool.tile([P, NDT, JC], BF16, tag="wk")
        wv = cpool.tile([P, NDT, JC], BF16, tag="wv")
        wo = cpool.tile([P, 2, D], BF16, tag="wo")
        sinf = cpool.tile([P, SL], F32, tag="sinf")
        cosf = cpool.tile([P, SL], F32, tag="cosf")
        mask = cpool.tile([P, P], BF16, tag="mask")
        QT = cpool.tile([P, 2, SL], BF16, tag="QT")     # plane 0: first halves
        KT = cpool.tile([P, 2, SL], BF16, tag="KT")
        # head-contiguous copies: plane p holds heads 2p (parts 0-63) and
        # 2p+1 (parts 64-127), dims 0-31 = rotated first half, 32-63 second
        QTc = cpool.tile([P, 2, SL], BF16, tag="QTc")
        KTc = cpool.tile([P, 2, SL], BF16, tag="KTc")
        V = cpool.tile([P, NST, HPC, 2 * HD], BF16, tag="V")
        ctxA = cpool.tile([P, SL], BF16, tag="ctxA")     # heads 0,1 (j on partitions)
        ctxB = cpool.tile([P, SL], BF16, tag="ctxB")     # heads 2,3

        # ---- input DMAs ----
        def xt_load(qc_i):
            for dt_i in range(NDT):
                nc.sync.dma_start(
                    xT[:, dt_i, qc_i * 512:(qc_i + 1) * 512],
                    xT_d[dt_i * P:(dt_i + 1) * P, qc_i * 512:(qc_i + 1) * 512])

        nc.sync.dma_start(wq[:], wq_d.rearrange("(t p) j -> p t j", p=P))
        nc.sync.dma_start(wk[:], wk_d.rearrange("(t p) j -> p t j", p=P))
        nc.sync.dma_start(sinf[:], sin_d[:])
        nc.sync.dma_start(cosf[:], cos_d[:])
        nc.sync.dma_start(wv[:], wv_d.rearrange("(t p) j -> p t j", p=P))
        nc.sync.dma_start(mask[:], mask_d[:])
        for _qc in range(min(3, NQC)):
            xt_load(_qc)
        nc.sync.dma_start(wo[:], wo_d.rearrange("(t p) n -> p t n", p=P))
        nc.gpsimd.memset(V[:, :, :, 0:HD], 1.0)  # denominator ones columns (base-0)

        # ---- filler queue: emission units interleaved into attention ----
        from collections import deque
        fillq = deque()

        def fill(n):
            for _ in range(n):
                if not fillq:
                    return
                fillq.popleft()[1]()

        def drain(gen):
            while fillq and fillq[0][0] <= gen:
                fillq.popleft()[1]()

        def qk_proj_units(qc, w_sb, out_sb):
            """Project + RoPE one 512-col s-chunk of QT or KT, as 6 units."""
            sl = slice(qc * 512, qc * 512 + 512)
            state = {}

            def pp():
                if "pp" not in state:
                    state["pp"] = ps_pr.tile([P, 2, 512], F32, tag="pr", name="qkproj")
                return state["pp"]

            def mm4(jt, hi):
                def run():
                    p = pp()
                    for dt_i in range(4 * hi, 4 * hi + 4):
                        nc.tensor.matmul(
                            p[:, jt, :],
                            lhsT=w_sb[:, dt_i, jt * P:(jt + 1) * P],
                            rhs=xT[:, dt_i, sl],
                            start=(dt_i == 0), stop=(dt_i == NDT - 1),
                        )
                return run

            dst = QTc if out_sb is QT else KTc

            def rope(half):
                def run():
                    p = pp()
                    pA, pB = p[:, 0, :], p[:, 1, :]
                    csl, ssl = cosf[:, sl], sinf[:, sl]
                    ta = tpool.tile([P, 512], F32, tag="t1")
                    tb = tpool.tile([P, 512], F32, tag="t2")
                    if half == 0:
                        nc.vector.tensor_tensor(ta[:], pA, csl, ALU.mult)
                        nc.vector.tensor_tensor(tb[:], pB, ssl, ALU.mult)
                        nc.vector.tensor_tensor(out_sb[:, 0, sl], ta[:], tb[:],
                                                ALU.subtract)
                    else:
                        nc.vector.tensor_tensor(ta[:], pA, ssl, ALU.mult)
                        nc.vector.tensor_tensor(tb[:], pB, csl, ALU.mult)
                        nc.vector.tensor_tensor(out_sb[:, 1, sl], ta[:], tb[:],
                                                ALU.add)
                    for h in range(HPC):
                        nc.sync.dma_start(
                            dst[64 * (h % 2) + 32 * half:
                                64 * (h % 2) + 32 * half + 32, h // 2, sl],
                            out_sb[32 * h:32 * h + 32, half, sl])
                return run

            return [mm4(0, 0), mm4(0, 1), mm4(1, 0), mm4(1, 1),
                    rope(0), rope(1)]

        def v_proj_units(st):
            """Project one 128-row seq tile of V, as 3 units."""
            state = {}

            def pp():
                if "pp" not in state:
                    state["pp"] = ps_pr.tile([P, 512], F32, tag="pr",
                                             name="vproj")
                return state["pp"]

            def mm4(hi):
                def run():
                    p = pp()[:, :JC]
                    for dt_i in range(4 * hi, 4 * hi + 4):
                        nc.tensor.matmul(
                            p[:],
                            lhsT=xT[:, dt_i, st * P:(st + 1) * P],
                            rhs=wv[:, dt_i, :],
                            start=(dt_i == 0), stop=(dt_i == NDT - 1),
                        )
                return run

            def copy():
                nc.vector.tensor_copy(
                    out=V[:, st, :, HD:2 * HD],
                    in_=pp()[:, :JC].rearrange("p (h d) -> p h d", h=HPC),
                )

            return [mm4(0), mm4(1), copy]

        def push_proj(qc, gen):
            for u in qk_proj_units(qc, wq, QT):
                fillq.append((gen, u))
            for u in qk_proj_units(qc, wk, KT):
                fillq.append((gen, u))
            for st in range(4 * qc, 4 * qc + 4):
                for u in v_proj_units(st):
                    fillq.append((gen, u))

        def run_proj(qc):
            for u in qk_proj_units(qc, wq, QT):
                u()
            for u in qk_proj_units(qc, wk, KT):
                u()
            for st in range(4 * qc, 4 * qc + 4):
                for u in v_proj_units(st):
                    u()

        def attention(qc, pair):
            """Causal flash attention for one q-chunk and one head pair.
            pair == plane index of QTc/KTc (heads 2*pair, 2*pair+1)."""
            heads = (2 * pair, 2 * pair + 1)
            n_ki = 4 * qc + 4
            ctx_ps = ps_ctx.tile([P, 2, 512], F32, tag="ctx")

            def sc_mm(ki):
                diag_r = ki - 4 * qc
                c0 = 128 * diag_r if diag_r >= 0 else 0
                nv = 512 - c0
                qsl = slice(qc * 512 + c0, qc * 512 + 512)
                st_ps = ps_sc.tile([P, 2, 512], F32, tag="sc")
                for hh in range(2):
                    pb = 64 * hh
                    nc.tensor.matmul(
                        st_ps[:, hh, :nv],
                        lhsT=KTc[pb:pb + 64, pair, ki * P:(ki + 1) * P],
                        rhs=QTc[pb:pb + 64, pair, qsl],
                        start=True, stop=True,
                        tile_position=(pb, 0),
                    )
                return st_ps, c0, nv

            pend = None
            for ki in range(n_ki):
                if pend is None:
                    pend = sc_mm(ki)
                st_ps, c0, nv = pend
                et = epool.tile([P, 2, 512], BF16, tag="expT")
                nc.scalar.activation(et[:, :, :nv], st_ps[:, :, :nv], AF.Exp,
                                     scale=float(scale))
                # prefetch next scores while exp runs
                pend = sc_mm(ki + 1) if ki + 1 < n_ki else None
                if c0 or ki == 4 * qc:  # diagonal block: mask upper triangle
                    for hh in range(2):
                        nc.gpsimd.tensor_tensor(et[:, hh, 0:P], et[:, hh, 0:P],
                                                mask[:], ALU.mult)
                for hh, h in enumerate(heads):
                    nc.tensor.matmul(
                        ctx_ps[:, hh, c0:512],
                        lhsT=V[:, ki, h, :],
                        rhs=et[:, hh, :nv],
                        start=(ki == 0), stop=(ki == n_ki - 1),
                    )
                fill(2)
            # normalize: psum rows 64-127 all hold the denominator
            for hh, h in enumerate(heads):
                denb = tpool.tile([HD, 512], F32, tag="denb")
                if RECIP == "fast":
                    # custom DVE op requires partition base 0 on both APs
                    nc.vector.reciprocal_approx_fast(out=denb[:],
                                                     in_=ctx_ps[0:HD, hh, :])
                else:
                    nc.vector.reciprocal(denb[:], ctx_ps[0:HD, hh, :])
                dst = ctxA if h < 2 else ctxB
                nc.vector.tensor_tensor(
                    dst[HD * (h % 2):HD * (h % 2) + HD, qc * 512:qc * 512 + 512],
                    ctx_ps[HD:2 * HD, hh, :], denb[:], ALU.mult)

        def out_proj(st):
            pp = ps_pr.tile([P, 2, 512], F32, tag="pr", name="oproj")
            for nh in range(2):
                for jt, csb in enumerate((ctxA, ctxB)):
                    nc.tensor.matmul(
                        pp[:, nh, :],
                        lhsT=csb[:, st * P:(st + 1) * P],
                        rhs=wo[:, jt, nh * 512:nh * 512 + 512],
                        start=(jt == 0), stop=(jt == 1),
                    )
            ot = tpool.tile([P, 2, 512], BF16, tag="ostage")
            nc.vector.tensor_copy(out=ot[:], in_=pp[:])
            nc.sync.dma_start(out_d[st * P:(st + 1) * P, :].rearrange(
                "p (t n) -> p t n", t=2), ot[:])

        # ---- emission (priority) order: software pipeline ----
        # prologue: projections for qc 0 and 1 run dense; later chunks are
        # pushed as filler units drained inside the attention windows
        for _qc in range(min(2, NQC)):
            run_proj(_qc)
        if NQC > 3:
            xt_load(3)
        for qc in range(NQC):
            drain(qc)
            if qc + 2 < NQC:
                push_proj(qc + 2, gen=qc + 2)
            attention(qc, 0)
            attention(qc, 1)
            for st in range(4 * qc, 4 * qc + 4):
                out_proj(st)
                fill(2)
        drain(NQC)


# ----------------------------------------------------------------------------
# host side
# ----------------------------------------------------------------------------

def _rope_tables(s_len):
    pos = np.arange(s_len, dtype=np.float32)
    inv_freq = np.exp(np.arange(0, HD, 2, dtype=np.float32)
                      * (-np.log(10000.0) / HD)).astype(np.float32)
    ang = pos[:, None] * inv_freq[None, :]          # [S, 32]
    sin = np.sin(ang).astype(np.float32)
    cos = np.cos(ang).astype(np.float32)
    # [128, S]: row 32h + i = table for freq i, replicated over the 4 heads
    sinf = np.ascontiguousarray(np.tile(sin.T, (HPC, 1)))
    cosf = np.ascontiguousarray(np.tile(cos.T, (HPC, 1)))
    return sinf, cosf


def _half_perm():
    """Column permutation grouping first/second halves of the 4 heads."""
    first = [64 * h + d for h in range(HPC) for d in range(32)]
    second = [64 * h + d for h in range(HPC) for d in range(32, 64)]
    return np.array(first + second, dtype=np.int64)


def build_program(s_len=S):
    nc = bacc.Bacc("TRN2", target_bir_lowering=False, debug=False,
                   num_devices=NCORES)
    io = {
        "xT": nc.dram_tensor("xT", [D, s_len], BF16, kind="ExternalInput").ap(),
        "wq": nc.dram_tensor("wq", [D, JC], BF16, kind="ExternalInput").ap(),
        "wk": nc.dram_tensor("wk", [D, JC], BF16, kind="ExternalInput").ap(),
        "wv": nc.dram_tensor("wv", [D, JC], BF16, kind="ExternalInput").ap(),
        "wo": nc.dram_tensor("wo", [JC, D], BF16, kind="ExternalInput").ap(),
        "sin": nc.dram_tensor("sin", [P, s_len], F32, kind="ExternalInput").ap(),
        "cos": nc.dram_tensor("cos", [P, s_len], F32, kind="ExternalInput").ap(),
        "mask": nc.dram_tensor("mask", [P, P], BF16, kind="ExternalInput").ap(),
        "out": nc.dram_tensor("out", [s_len, D], BF16, kind="ExternalOutput").ap(),
    }
    with tile.TileContext(nc) as tc:
        build_core(tc, io, s_len)
    nc.compile()
    return nc


def make_in_maps(x, Wq, Wk, Wv, Wo, s_len=S):
    """Shard the full inputs into one input map per core."""
    bf16 = ml_dtypes.bfloat16
    perm = _half_perm()
    sinf, cosf = _rope_tables(s_len)
    mask = np.triu(np.ones((P, P), dtype=np.float32)).astype(bf16)
    in_maps = []
    for c in range(NCORES):
        b, g = divmod(c, NCORES // B)
        cols = slice(JC * g, JC * (g + 1))
        in_maps.append({
            "xT": np.ascontiguousarray(x[b].T).astype(bf16),
            "wq": np.ascontiguousarray(Wq[:, cols][:, perm]).astype(bf16),
            "wk": np.ascontiguousarray(Wk[:, cols][:, perm]).astype(bf16),
            "wv": np.ascontiguousarray(Wv[:, cols]).astype(bf16),
            "wo": np.ascontiguousarray(Wo[cols, :]).astype(bf16),
            "sin": sinf, "cos": cosf, "mask": mask,
        })
    return in_maps


_CACHED_NC = None


def kernel(x, Wq, bq, Wk, bk, Wv, bv, Wo, bo, **run_kwargs):
    global _CACHED_NC
    x, Wq, bq, Wk, bk, Wv, bv, Wo, bo = (
        np.asarray(a, dtype=np.float32)
        for a in (x, Wq, bq, Wk, bk, Wv, bv, Wo, bo))
    assert not (np.any(bq) or np.any(bk) or np.any(bv)), \
        "nonzero qkv biases not supported by this build"
    if _CACHED_NC is None:
        _CACHED_NC = build_program(S)
    in_maps = make_in_maps(x, Wq, Wk, Wv, Wo, S)
    res = run_bass_kernel_spmd(_CACHED_NC, in_maps, list(range(NCORES)),
                               **run_kwargs)
    out = np.zeros((B, S, D), dtype=np.float32)
    for c in range(NCORES):
        b = c // (NCORES // B)
        out[b] += res.results[c]["out"].astype(np.float32)
    out += bo[None, None, :]
    if run_kwargs:
        kernel.last_result = res
    return out


# revision 23
# speedup vs baseline: 1.0975x; 1.0975x over previous
"""Trainium2 Bass kernel for CustomMultiHeadAttention (RoPE + causal MHA).

Sharding: 8 cores = 2 batches x 4 head-groups (4 heads each).
Each core computes, for its (batch, head-group):
  QT/KT = (Wq|Wk col-slice, half-permuted).T @ xT   -> [256, S] feature-major
  RoPE on QT/KT (full-tile DVE ops thanks to half-grouped layout)
  V = xT.T @ Wv col-slice                            -> [S, 256] natural
  per head pair: scoresT[k,q] = KT_h.T @ QT_h (quadrant-packed, K=64)
            expT = exp(scoresT/8), one ACTIVATE covering both heads
            causal: skip blocks above diagonal, 0/1-mask diagonal blocks
            ctxT[d,q] (+denominator rows via ones-columns in V_aug) = V_aug.T @ expT
  normalize ctxT with reciprocal_approx_fast + mult
  partial_out = ctxT.T @ Wo row-slice               -> [S, 1024] bf16
Host: sums the 4 head-group partials per batch, adds bo.

All DRAM traffic is bf16 (x, weights, partial outputs); sin/cos fp32.
Emission is software-pipelined: projections for chunk qc+2 are emitted
between the two attention pairs of chunk qc so the PE always has dense
matmul work while ACT runs exp and DVE runs RoPE/normalize.
"""

import os
import sys

for _p in ("/opt/trn_rl_repo", "/root/.axon_site/_ro/trn_rl_repo"):
    if os.path.isdir(_p) and _p not in sys.path:
        sys.path.insert(0, _p)

import numpy as np
import ml_dtypes

import concourse.bass as bass
import concourse.bacc as bacc
import concourse.mybir as mybir
import concourse.tile as tile
from concourse.bass_utils import run_bass_kernel_spmd

F32 = mybir.dt.float32
BF16 = mybir.dt.bfloat16
FP8 = mybir.dt.float8e4
AF = mybir.ActivationFunctionType
ALU = mybir.AluOpType

NUM_HEADS = 16
HD = 64
D = NUM_HEADS * HD  # 1024
B = 2
S = 2048
NCORES = 8
HPC = 4            # heads per core
JC = HPC * HD      # 256 per-core projection width
P = 128

RECIP = os.environ.get("KERNEL_RECIP", "fast")   # fast | exact
EXP_BIAS = -2.0     # keeps exp outputs in fp8e4 range; cancels in softmax
VSCALE = 16.0       # fp8 V scale; ones=16 too, cancels in normalize


def build_core(tc, io, s_len=S):
    """Emit the per-core program."""
    nc = tc.nc
    SL = s_len
    NST = SL // P          # 128-row seq tiles
    NQC = SL // 512        # 512-wide q chunks
    NDT = D // P           # 8 k-tiles over d_model
    scale = 1.0 / np.sqrt(HD)

    xT_d, wq_d, wk_d, wv_d, wo_d = io["xT"], io["wq"], io["wk"], io["wv"], io["wo"]
    sin_d, cos_d, mask_d, out_d = io["sin"], io["cos"], io["mask"], io["out"]

    import contextlib
    with contextlib.ExitStack() as ctx:
        cpool = ctx.enter_context(tc.tile_pool(name="const", bufs=1))
        epool = ctx.enter_context(tc.tile_pool(name="expt", bufs=6))
        tpool = ctx.enter_context(tc.tile_pool(name="tmps", bufs=4))
        # PSUM: 8 banks total = sc(2x2) + ctx(1x2) + pr(1x2)
        ps_sc = ctx.enter_context(tc.tile_pool(name="ps_sc", bufs=2, space="PSUM"))
        ps_ctx = ctx.enter_context(tc.tile_pool(name="ps_ctx", bufs=1, space="PSUM"))
        ps_pr = ctx.enter_context(tc.tile_pool(name="ps_pr", bufs=1, space="PSUM"))

        # ---- persistent SBUF tensors ----
        xT = cpool.tile([P, NDT, SL], BF16, tag="xT")
        wq = cpool.tile([P, NDT, JC], BF16, tag="wq")
        wk = cpool.tile([P, NDT, JC], BF16, tag="wk")
        wv = cpool.tile([P, NDT, JC], BF16, tag="wv")
        wo = cpool.tile([P, 2, D], BF16, tag="wo")
        sinf = cpool.tile([P, SL], BF16, tag="sinf")
        cosf = cpool.tile([P, SL], BF16, tag="cosf")
        mask = cpool.tile([P, P], BF16, tag="mask")
        QT = cpool.tile([P, 2, SL], BF16, tag="QT")     # plane 0: first halves
        KT = cpool.tile([P, 2, SL], BF16, tag="KT")
        # head-contiguous copies: plane p holds heads 2p (parts 0-63) and
        # 2p+1 (parts 64-127), dims 0-31 = rotated first half, 32-63 second
        QTc = cpool.tile([P, 2, SL], BF16, tag="QTc")
        KTc = cpool.tile([P, 2, SL], BF16, tag="KTc")
        V = cpool.tile([P, NST, HPC, 2 * HD], BF16, tag="V")
        ctxA = cpool.tile([P, SL], BF16, tag="ctxA")     # heads 0,1 (j on partitions)
        ctxB = cpool.tile([P, SL], BF16, tag="ctxB")     # heads 2,3

        # ---- input DMAs ----
        xT_dv = xT_d.rearrange("(t p) s -> p t s", p=P)

        def xt_load(qc_i, split=True):
            sl = slice(qc_i * 512, (qc_i + 1) * 512)
            for dt_i in range(NDT):
                nc.sync.dma_start(
                    xT[:, dt_i, sl],
                    xT_d[dt_i * P:(dt_i + 1) * P, sl])

        nc.sync.dma_start(wq[:], wq_d.rearrange("(t p) j -> p t j", p=P))
        xt_load(0)
        nc.sync.dma_start(wk[:], wk_d.rearrange("(t p) j -> p t j", p=P))
        nc.sync.dma_start(sinf[:], sin_d[:])
        nc.sync.dma_start(cosf[:], cos_d[:])
        nc.sync.dma_start(wv[:], wv_d.rearrange("(t p) j -> p t j", p=P))
        nc.sync.dma_start(mask[:], mask_d[:])
        nc.sync.dma_start(wo[:], wo_d.rearrange("(t p) n -> p t n", p=P))
        nc.gpsimd.memset(V[:, :, :, 0:HD], 1.0)  # den ones columns (base-0)

        # ---- filler queue: emission units interleaved into attention ----
        import heapq
        fillq = []
        fillseq = [0]

        def push_unit(gen, u):
            heapq.heappush(fillq, (gen, fillseq[0], u))
            fillseq[0] += 1

        def fill(n):
            for _ in range(n):
                if not fillq:
                    return
                heapq.heappop(fillq)[2]()

        def drain(gen):
            while fillq and fillq[0][0] <= gen:
                heapq.heappop(fillq)[2]()

        def qk_proj_units(qc, w_sb, out_sb, pool=None, tag="pr", dma=None):
            """Project + RoPE one 512-col s-chunk of QT or KT, as 6 units."""
            sl = slice(qc * 512, qc * 512 + 512)
            pool = pool if pool is not None else ps_pr
            dma = dma if dma is not None else nc.sync
            state = {}

            def pp():
                if "pp" not in state:
                    state["pp"] = pool.tile([P, 2, 512], F32, tag=tag,
                                            name="qkproj")
                return state["pp"]

            def mm4(jt, hi):
                def run():
                    p = pp()
                    for dt_i in range(4 * hi, 4 * hi + 4):
                        nc.tensor.matmul(
                            p[:, jt, :],
                            lhsT=w_sb[:, dt_i, jt * P:(jt + 1) * P],
                            rhs=xT[:, dt_i, sl],
                            start=(dt_i == 0), stop=(dt_i == NDT - 1),
                        )
                return run

            dst = QTc if out_sb is QT else KTc

            def rope(half):
                def run():
                    p = pp()
                    pA, pB = p[:, 0, :], p[:, 1, :]
                    csl, ssl = cosf[:, sl], sinf[:, sl]
                    ta = tpool.tile([P, 512], F32, tag="t1")
                    tb = tpool.tile([P, 512], F32, tag="t2")
                    if half == 0:
                        nc.vector.tensor_tensor(ta[:], pA, csl, ALU.mult)
                        nc.vector.tensor_tensor(tb[:], pB, ssl, ALU.mult)
                        nc.vector.tensor_tensor(out_sb[:, 0, sl], ta[:], tb[:],
                                                ALU.subtract)
                    else:
                        nc.vector.tensor_tensor(ta[:], pA, ssl, ALU.mult)
                        nc.vector.tensor_tensor(tb[:], pB, csl, ALU.mult)
                        nc.vector.tensor_tensor(out_sb[:, 1, sl], ta[:], tb[:],
                                                ALU.add)
                    for h in range(HPC):
                        dma.dma_start(
                            dst[64 * (h % 2) + 32 * half:
                                64 * (h % 2) + 32 * half + 32, h // 2, sl],
                            out_sb[32 * h:32 * h + 32, half, sl])
                return run

            return [mm4(0, 0), mm4(0, 1), mm4(1, 0), mm4(1, 1),
                    rope(0), rope(1)]

        def v_proj_units(st, pool=None, tag="pr"):
            """Project one 128-row seq tile of V, as 3 units."""
            pool = pool if pool is not None else ps_pr
            state = {}

            def pp():
                if "pp" not in state:
                    state["pp"] = pool.tile([P, 512], F32, tag=tag,
                                            name="vproj")
                return state["pp"]

            def mm4(hi):
                def run():
                    p = pp()[:, :JC]
                    for dt_i in range(4 * hi, 4 * hi + 4):
                        nc.tensor.matmul(
                            p[:],
                            lhsT=xT[:, dt_i, st * P:(st + 1) * P],
                            rhs=wv[:, dt_i, :],
                            start=(dt_i == 0), stop=(dt_i == NDT - 1),
                        )
                return run

            def copy():
                nc.vector.tensor_copy(
                    out=V[:, st, :, HD:2 * HD],
                    in_=pp()[:, :JC].rearrange("p (h d) -> p h d", h=HPC),
                )

            return [mm4(0), mm4(1), copy]

        def push_proj(qc, gen):
            for u in qk_proj_units(qc, wq, QT):
                push_unit(gen, u)
            for u in qk_proj_units(qc, wk, KT):
                push_unit(gen, u)
            for st in range(4 * qc, 4 * qc + 4):
                for u in v_proj_units(st):
                    push_unit(gen, u)

        def run_proj(qc):
            # prologue: run from the (idle) scores pool so two chains
            # pipeline, and issue shuffle DMAs from the (idle) scalar queue
            for u in qk_proj_units(qc, wq, QT, pool=ps_sc, tag="sc",
                                   dma=nc.scalar):
                u()
            for u in qk_proj_units(qc, wk, KT, pool=ps_sc, tag="sc",
                                   dma=nc.scalar):
                u()
            for st in range(4 * qc, 4 * qc + 4):
                for u in v_proj_units(st, pool=ps_sc, tag="sc"):
                    u()

        def sc_mm(qc, pair, ki):
            diag_r = ki - 4 * qc
            c0 = 128 * diag_r if diag_r >= 0 else 0
            nv = 512 - c0
            qsl = slice(qc * 512 + c0, qc * 512 + 512)
            st_ps = ps_sc.tile([P, 2, 512], F32, tag="sc")
            for hh in range(2):
                pb = 64 * hh
                nc.tensor.matmul(
                    st_ps[:, hh, :nv],
                    lhsT=KTc[pb:pb + 64, pair, ki * P:(ki + 1) * P],
                    rhs=QTc[pb:pb + 64, pair, qsl],
                    start=True, stop=True,
                    tile_position=(pb, 0),
                )
            return st_ps, c0, nv

        def attention(qc, pair, pend, nxt):
            """Causal flash attention for one q-chunk and one head pair.
            `pend` is this pair's prefetched first scores block (or None);
            returns the prefetched first block of `nxt` = (qc', pair')."""
            heads = (2 * pair, 2 * pair + 1)
            n_ki = 4 * qc + 4
            ctx_ps = ps_ctx.tile([P, 2, 512], F32, tag="ctx")

            if pend is None:
                pend = sc_mm(qc, pair, 0)
            for ki in range(n_ki):
                st_ps, c0, nv = pend
                et = epool.tile([P, 2, 512], BF16, tag="expT")
                nc.scalar.activation(et[:, :, :nv], st_ps[:, :, :nv], AF.Exp,
                                     scale=float(scale))
                # prefetch next scores (possibly the next pair's) while exp runs
                if ki + 1 < n_ki:
                    pend = sc_mm(qc, pair, ki + 1)
                elif nxt is not None:
                    pend = sc_mm(nxt[0], nxt[1], 0)
                else:
                    pend = None
                if c0 or ki == 4 * qc:  # diagonal block: mask upper triangle
                    for hh in range(2):
                        nc.gpsimd.tensor_tensor(et[:, hh, 0:P], et[:, hh, 0:P],
                                                mask[:], ALU.mult)
                for hh, h in enumerate(heads):
                    nc.tensor.matmul(
                        ctx_ps[:, hh, c0:512],
                        lhsT=V[:, ki, h, :],
                        rhs=et[:, hh, :nv],
                        start=(ki == 0), stop=(ki == n_ki - 1),
                    )
                if ki < n_ki - 2:   # keep DVE clear for the normalize
                    fill(2)
            # normalize: psum rows 64-127 all hold the denominator
            for hh, h in enumerate(heads):
                denb = tpool.tile([HD, 512], F32, tag="denb")
                if RECIP == "fast":
                    # custom DVE op requires partition base 0 on both APs
                    nc.vector.reciprocal_approx_fast(out=denb[:],
                                                     in_=ctx_ps[0:HD, hh, :])
                else:
                    nc.vector.reciprocal(denb[:], ctx_ps[0:HD, hh, :])
                dst = ctxA if h < 2 else ctxB
                nc.vector.tensor_tensor(
                    dst[HD * (h % 2):HD * (h % 2) + HD, qc * 512:qc * 512 + 512],
                    ctx_ps[HD:2 * HD, hh, :], denb[:], ALU.mult)
            fill(4)

        def out_proj_units(st, pool=None, tag="pr"):
            pool = pool if pool is not None else ps_pr
            state = {}

            def pp():
                if "pp" not in state:
                    state["pp"] = pool.tile([P, 2, 512], F32, tag=tag,
                                            name="oproj")
                return state["pp"]

            def mms():
                p = pp()
                for nh in range(2):
                    for jt, csb in enumerate((ctxA, ctxB)):
                        nc.tensor.matmul(
                            p[:, nh, :],
                            lhsT=csb[:, st * P:(st + 1) * P],
                            rhs=wo[:, jt, nh * 512:nh * 512 + 512],
                            start=(jt == 0), stop=(jt == 1),
                        )

            def store():
                ot = tpool.tile([P, 2, 512], BF16, tag="ostage")
                nc.vector.tensor_copy(out=ot[:], in_=pp()[:])
                nc.sync.dma_start(out_d[st * P:(st + 1) * P, :].rearrange(
                    "p (t n) -> p t n", t=2), ot[:])

            return [mms, store]

        # ---- emission (priority) order: software pipeline ----
        # prologue: only chunk 0's projections run dense; everything later
        # is pushed as filler units drained inside the attention windows
        run_proj(0)
        if NQC > 1:
            xt_load(1)
            for u in qk_proj_units(1, wq, QT, pool=ps_sc, tag="sc",
                                   dma=nc.scalar):
                u()
            for u in qk_proj_units(1, wk, KT, pool=ps_sc, tag="sc",
                                   dma=nc.scalar):
                u()
            for st in range(4, 8):
                for u in v_proj_units(st):
                    push_unit(1, u)
        pend = None
        for qc in range(NQC):
            if qc + 2 < NQC:
                xt_load(qc + 2)
                push_proj(qc + 2, gen=qc + 2)
            pend = attention(qc, 0, pend, (qc, 1))
            drain(qc + 1)   # next window's projections fully emitted now
            nxt0 = (qc + 1, 0) if qc + 1 < NQC else None
            pend = attention(qc, 1, pend, nxt0)
            if qc + 1 < NQC:
                for st in range(4 * qc, 4 * qc + 4):
                    for u in out_proj_units(st):
                        push_unit(qc + 1, u)
            else:
                for st in range(4 * qc, 4 * qc + 4):
                    for u in out_proj_units(st, pool=ps_sc, tag="sc"):
                        u()
        drain(NQC)


# ----------------------------------------------------------------------------
# host side
# ----------------------------------------------------------------------------

def _rope_tables(s_len):
    pos = np.arange(s_len, dtype=np.float32)
    inv_freq = np.exp(np.arange(0, HD, 2, dtype=np.float32)
                      * (-np.log(10000.0) / HD)).astype(np.float32)
    ang = pos[:, None] * inv_freq[None, :]          # [S, 32]
    sin = np.sin(ang).astype(np.float32)
    cos = np.cos(ang).astype(np.float32)
    # [128, S]: row 32h + i = table for freq i, replicated over the 4 heads
    sinf = np.ascontiguousarray(np.tile(sin.T, (HPC, 1))).astype(ml_dtypes.bfloat16)
    cosf = np.ascontiguousarray(np.tile(cos.T, (HPC, 1))).astype(ml_dtypes.bfloat16)
    return sinf, cosf


def _half_perm():
    """Column permutation grouping first/second halves of the 4 heads."""
    first = [64 * h + d for h in range(HPC) for d in range(32)]
    second = [64 * h + d for h in range(HPC) for d in range(32, 64)]
    return np.array(first + second, dtype=np.int64)


def build_program(s_len=S):
    nc = bacc.Bacc("TRN2", target_bir_lowering=False, debug=False,
                   num_devices=NCORES)
    io = {
        "xT": nc.dram_tensor("xT", [D, s_len], BF16, kind="ExternalInput").ap(),
        "wq": nc.dram_tensor("wq", [D, JC], BF16, kind="ExternalInput").ap(),
        "wk": nc.dram_tensor("wk", [D, JC], BF16, kind="ExternalInput").ap(),
        "wv": nc.dram_tensor("wv", [D, JC], BF16, kind="ExternalInput").ap(),
        "wo": nc.dram_tensor("wo", [JC, D], BF16, kind="ExternalInput").ap(),
        "sin": nc.dram_tensor("sin", [P, s_len], BF16, kind="ExternalInput").ap(),
        "cos": nc.dram_tensor("cos", [P, s_len], BF16, kind="ExternalInput").ap(),
        "mask": nc.dram_tensor("mask", [P, P], BF16, kind="ExternalInput").ap(),
        "out": nc.dram_tensor("out", [s_len, D], BF16, kind="ExternalOutput").ap(),
    }
    with tile.TileContext(nc) as tc:
        build_core(tc, io, s_len)
    nc.compile()
    return nc


def make_in_maps(x, Wq, Wk, Wv, Wo, s_len=S):
    """Shard the full inputs into one input map per core."""
    bf16 = ml_dtypes.bfloat16
    perm = _half_perm()
    sinf, cosf = _rope_tables(s_len)
    mask = np.triu(np.ones((P, P), dtype=np.float32)).astype(bf16)
    in_maps = []
    for c in range(NCORES):
        b, g = divmod(c, NCORES // B)
        cols = slice(JC * g, JC * (g + 1))
        in_maps.append({
            "xT": np.ascontiguousarray(x[b].T).astype(bf16),
            "wq": np.ascontiguousarray(Wq[:, cols][:, perm]).astype(bf16),
            "wk": np.ascontiguousarray(Wk[:, cols][:, perm]).astype(bf16),
            "wv": np.ascontiguousarray(Wv[:, cols]).astype(bf16),
            "wo": np.ascontiguousarray(Wo[cols, :]).astype(bf16),
            "sin": sinf, "cos": cosf, "mask": mask,
        })
    return in_maps


_CACHED_NC = None


def kernel(x, Wq, bq, Wk, bk, Wv, bv, Wo, bo, **run_kwargs):
    global _CACHED_NC
    x, Wq, bq, Wk, bk, Wv, bv, Wo, bo = (
        np.asarray(a, dtype=np.float32)
        for a in (x, Wq, bq, Wk, bk, Wv, bv, Wo, bo))
    assert not (np.any(bq) or np.any(bk) or np.any(bv)), \
        "nonzero qkv biases not supported by this build"
    if _CACHED_NC is None:
        _CACHED_NC = build_program(S)
    in_maps = make_in_maps(x, Wq, Wk, Wv, Wo, S)
    res = run_bass_kernel_spmd(_CACHED_NC, in_maps, list(range(NCORES)),
                               **run_kwargs)
    out = np.zeros((B, S, D), dtype=np.float32)
    for c in range(NCORES):
        b = c // (NCORES // B)
        out[b] += res.results[c]["out"].astype(np.float32)
    out += bo[None, None, :]
    if run_kwargs:
        kernel.last_result = res
    return out
